# revision 10
# baseline (speedup 1.0000x reference)
"""Trainium2 Bass kernel for nn_Attention_Encode (B=4, N=2048, DIM=1024, H=16, DH=64).

Sharding: 16 heads -> 8 cores x 2 heads (tensor parallel). Each core computes
  ztu_g = W_g @ ZT^T          (its 128 output channels = 2 heads)
  attention per (batch, head) with Q=K=V=ztu
  partial_out = ssa_g @ W_g   (row-sharded output projection)
Host sums the 8 partials (the all-reduce step of a row-sharded projection).

On-device layout is fully transposed ("scoresT" = [keys, queries]) so that
softmax needs no transposes: the AV matmul's stationary operand [V | ones]
produces both the numerator and the softmax denominator.
"""
import sys

for _p in ('/opt/trn_rl_repo',):
    if _p not in sys.path:
        sys.path.insert(0, _p)

from contextlib import ExitStack

import numpy as np
import ml_dtypes

import concourse.bacc as bacc
import concourse.mybir as mybir
import concourse.tile as tile
from concourse.bass_utils import run_bass_kernel_spmd
from concourse.masks import make_identity

B, N, C = 4, 2048, 1024          # batch, seq, model dim
KP, DH, HPER = 128, 64, 2        # per-core channels, head dim, heads per core
NQB = 512                        # query block
NKT = 128                        # key tile
NTB = N // NKT                   # 16 key tiles per batch
NTILES = B * NTB                 # 64 n-tiles total
SCALE = DH ** -0.5               # 0.125
BF = mybir.dt.bfloat16
F32 = mybir.dt.float32

_CACHE = {}


def _build_kernel():
    nc = bacc.Bacc("TRN2", target_bir_lowering=False, debug=False)
    ztt = nc.dram_tensor("ztt", [B, C, N], BF, kind="ExternalInput").ap()
    wgt = nc.dram_tensor("wgt", [C, KP], BF, kind="ExternalInput").ap()   # W_g^T
    wg = nc.dram_tensor("wg", [KP, C], BF, kind="ExternalInput").ap()     # W_g
    out = nc.dram_tensor("out", [B * N, C], F32, kind="ExternalOutput").ap()

    with tile.TileContext(nc) as tc, ExitStack() as ctx:
        _body(ctx, tc, ztt, wgt, wg, out)
    nc.compile()
    return nc


def _body(ctx, tc, ztt, wgt, wg, out):
    nc = tc.nc
    singles = ctx.enter_context(tc.tile_pool(name="singles", bufs=1))
    zin_pool = ctx.enter_context(tc.tile_pool(name="zin", bufs=8))
    sc_pool = ctx.enter_context(tc.tile_pool(name="sc", bufs=2, space="PSUM"))
    av_pool = ctx.enter_context(tc.tile_pool(name="av", bufs=2, space="PSUM"))
    p2_pool = ctx.enter_context(tc.tile_pool(name="p2", bufs=2, space="PSUM"))
    ex_pool = ctx.enter_context(tc.tile_pool(name="ex", bufs=6))
    sn_pool = ctx.enter_context(tc.tile_pool(name="sn", bufs=3))
    rc_pool = ctx.enter_context(tc.tile_pool(name="rc", bufs=3))

    # ---- persistent SBUF ----
    wgt_sb = singles.tile([128, 8, KP], BF)            # [c-in-tile, ci, k]
    nc.sync.dma_start(out=wgt_sb, in_=wgt.rearrange("(ci p) k -> p ci k", p=128))
    wg_sb = singles.tile([KP, C], BF)
    nc.sync.dma_start(out=wg_sb, in_=wg)
    ident = singles.tile([128, 128], BF)
    make_identity(nc, ident)
    sel = singles.tile([64, 128], F32)                 # den -> per-head row broadcast
    nc.vector.memset(sel, 0.0)
    nc.vector.memset(sel[0:1, 0:64], 1.0)
    nc.vector.memset(sel[32:33, 64:128], 1.0)
    dn = singles.tile([64, NQB], F32)                  # dens: head A row 0, head B row 32
    nc.vector.memset(dn, 0.0)
    ztuT = singles.tile([128, B * N], BF)              # [k-chan, b*N + n]
    ztuN = singles.tile([128, NTILES, 2 * (DH + 1)], BF)  # [n-in-tile, nt, [vA|1|vB|1]]
    nc.vector.memset(ztuN[:, :, DH:DH + 1], 1.0)
    nc.vector.memset(ztuN[:, :, 2 * DH + 1:2 * DH + 2], 1.0)

    # ---- phase 1: proj1 (ztuT = W_g @ ZT^T) + phase 1.5: transposes (ztuN) ----
    for b in range(B):
        zin = []
        for ci in range(8):
            z = zin_pool.tile([128, N], BF, tag="zin")
            nc.sync.dma_start(out=z, in_=ztt[b, ci * 128:(ci + 1) * 128, :])
            zin.append(z)
        for jn in range(N // NQB):
            p1 = sc_pool.tile([128, 2 * NQB], F32, tag="sc")
            p1v = p1[:, 0:NQB]
            for ci in range(8):
                nc.tensor.matmul(
                    p1v, lhsT=wgt_sb[:, ci, :],
                    rhs=zin[ci][:, jn * NQB:(jn + 1) * NQB],
                    start=(ci == 0), stop=(ci == 7),
                )
            nc.vector.tensor_copy(
                out=ztuT[:, b * N + jn * NQB: b * N + (jn + 1) * NQB], in_=p1v)
        for ntl in range(NTB):
            nt = b * NTB + ntl
            for hh in range(HPER):
                pt = av_pool.tile([128, NQB], BF, tag="av")
                nc.tensor.transpose(
                    pt[:, 0:DH],
                    ztuT[hh * DH:(hh + 1) * DH, nt * NKT:(nt + 1) * NKT],
                    ident[hh * DH:(hh + 1) * DH, hh * DH:(hh + 1) * DH],
                )
                nc.vector.tensor_copy(
                    out=ztuN[:, nt, hh * (DH + 1): hh * (DH + 1) + DH],
                    in_=pt[:, 0:DH])

    # ---- phase 2: attention + proj2, per (batch, q-block) ----
    for b in range(B):
        for jq in range(N // NQB):
            q0 = b * N + jq * NQB
            avs = []
            for hh in range(HPER):
                av = av_pool.tile([DH + 1, NQB], F32, tag="av")
                qT = ztuT[hh * DH:(hh + 1) * DH, q0:q0 + NQB]
                for g in range(NTB // 2):           # groups of 2 key tiles
                    sc = sc_pool.tile([128, 2 * NQB], F32, tag="sc")
                    ex = ex_pool.tile([128, 2 * NQB], BF, tag="ex")
                    for u in range(2):
                        ik = 2 * g + u
                        kT = ztuT[hh * DH:(hh + 1) * DH,
                                  b * N + ik * NKT: b * N + (ik + 1) * NKT]
                        nc.tensor.matmul(sc[:, u * NQB:(u + 1) * NQB],
                                         lhsT=kT, rhs=qT, start=True, stop=True)
                    nc.scalar.activation(
                        out=ex, in_=sc,
                        func=mybir.ActivationFunctionType.Exp, scale=SCALE)
                    for u in range(2):
                        ik = 2 * g + u
                        vT = ztuN[:, b * NTB + ik,
                                  hh * (DH + 1): (hh + 1) * (DH + 1)]
                        nc.tensor.matmul(av, lhsT=vT,
                                         rhs=ex[:, u * NQB:(u + 1) * NQB],
                                         start=(ik == 0), stop=(ik == NTB - 1))
                avs.append(av)

            # softmax denominators -> per-head broadcast -> reciprocal -> scale
            nc.vector.tensor_copy(out=dn[0:1, :], in_=avs[0][DH:DH + 1, :])
            nc.vector.tensor_copy(out=dn[32:33, :], in_=avs[1][DH:DH + 1, :])
            bc = sc_pool.tile([128, 2 * NQB], F32, tag="sc")
            nc.tensor.matmul(bc[:, 0:NQB], lhsT=sel, rhs=dn, start=True, stop=True)
            rc = rc_pool.tile([128, NQB], F32)
            nc.vector.reciprocal_approx_fast(out=rc, in_=bc[:, 0:NQB])
            sn = sn_pool.tile([128, NQB], BF)
            nc.vector.tensor_tensor(
                out=sn[0:64, :], in0=avs[0][0:DH, :], in1=rc[0:64, :],
                op=mybir.AluOpType.mult)
            nc.vector.tensor_tensor(
                out=sn[64:128, :], in0=avs[1][0:DH, :], in1=rc[64:128, :],
                op=mybir.AluOpType.mult)

            # proj2: out[q, :] += ssa_norm_g @ W_g  (both heads contracted)
            for t in range(NQB // 128):
                for ch in range(2):
                    p2 = p2_pool.tile([128, 512], F32)
                    nc.tensor.matmul(
                        p2, lhsT=sn[:, t * 128:(t + 1) * 128],
                        rhs=wg_sb[:, ch * 512:(ch + 1) * 512],
                        start=True, stop=True)
                    p2s = rc_pool.tile([128, 512], F32, tag="p2s")
                    nc.vector.tensor_copy(out=p2s, in_=p2)
                    r0 = b * N + jq * NQB + t * 128
                    nc.sync.dma_start(
                        out=out[r0:r0 + 128, ch * 512:(ch + 1) * 512], in_=p2s)


def _get_nc():
    if "nc" not in _CACHE:
        _CACHE["nc"] = _build_kernel()
    return _CACHE["nc"]


def kernel(ZT: np.ndarray, W: np.ndarray) -> np.ndarray:
    ZT = np.asarray(ZT, dtype=np.float32)
    W = np.asarray(W, dtype=np.float32)
    ztt = np.ascontiguousarray(ZT.transpose(0, 2, 1)).astype(ml_dtypes.bfloat16)
    in_maps = []
    for c in range(8):
        wgf = W[c * KP:(c + 1) * KP, :]
        in_maps.append({
            "ztt": ztt,
            "wgt": np.ascontiguousarray(wgf.T).astype(ml_dtypes.bfloat16),
            "wg": np.ascontiguousarray(wgf).astype(ml_dtypes.bfloat16),
        })
    nc = _get_nc()
    res = run_bass_kernel_spmd(nc, in_maps, core_ids=list(range(8)))
    acc = np.zeros((B * N, C), dtype=np.float32)
    for r in res.results:
        acc += r["out"]
    return acc.reshape(B, N, C)


if __name__ == "__main__":
    rng = np.random.default_rng(0)
    zt = rng.standard_normal((B, N, C), dtype=np.float32)
    w = rng.standard_normal((KP * 8, C), dtype=np.float32) * C ** -0.5
    o = kernel(zt, w)
    print("out", o.shape, o.dtype, float(np.abs(o).mean()))


# revision 12
# speedup vs baseline: 1.1698x; 1.1698x over previous
"""Trainium2 Bass kernel for nn_Attention_Encode (B=4, N=2048, DIM=1024, H=16, DH=64).

Sharding: 16 heads -> 8 cores x 2 heads (tensor parallel). Each core computes
  ztu_g = W_g @ ZT^T          (its 128 output channels = 2 heads)
  attention per (batch, head) with Q=K=V=ztu
  partial_out = ssa_g @ W_g   (row-sharded output projection)
Host sums the 8 partials (the all-reduce step of a row-sharded projection).

On-device layout is fully transposed ("scoresT" = [keys, queries]) so that
softmax needs no transposes: the AV matmul's stationary operand [V | ones]
produces both the numerator and the softmax denominator.
"""
import sys

for _p in ('/opt/trn_rl_repo',):
    if _p not in sys.path:
        sys.path.insert(0, _p)

from contextlib import ExitStack

import numpy as np
import ml_dtypes

import concourse.bacc as bacc
import concourse.mybir as mybir
import concourse.tile as tile
from concourse.bass_utils import run_bass_kernel_spmd
from concourse.masks import make_identity

B, N, C = 4, 2048, 1024          # batch, seq, model dim
KP, DH, HPER = 128, 64, 2        # per-core channels, head dim, heads per core
NQB = 512                        # query block
NKT = 128                        # key tile
NTB = N // NKT                   # 16 key tiles per batch
NTILES = B * NTB                 # 64 n-tiles total
SCALE = DH ** -0.5               # 0.125
BF = mybir.dt.bfloat16
F32 = mybir.dt.float32

_CACHE = {}


def _build_kernel():
    nc = bacc.Bacc("TRN2", target_bir_lowering=False, debug=False)
    ztt = nc.dram_tensor("ztt", [B, C, N], BF, kind="ExternalInput").ap()
    wgt = nc.dram_tensor("wgt", [C, KP], BF, kind="ExternalInput").ap()   # W_g^T
    wg = nc.dram_tensor("wg", [KP, C], BF, kind="ExternalInput").ap()     # W_g
    out = nc.dram_tensor("out", [B * N, C], F32, kind="ExternalOutput").ap()

    with tile.TileContext(nc) as tc, ExitStack() as ctx:
        _body(ctx, tc, ztt, wgt, wg, out)
    nc.compile()
    return nc


def _body(ctx, tc, ztt, wgt, wg, out):
    nc = tc.nc
    singles = ctx.enter_context(tc.tile_pool(name="singles", bufs=1))
    zin_pool = ctx.enter_context(tc.tile_pool(name="zin", bufs=8))
    sc_pool = ctx.enter_context(tc.tile_pool(name="sc", bufs=2, space="PSUM"))
    av_pool = ctx.enter_context(tc.tile_pool(name="av", bufs=4, space="PSUM"))
    ex_pool = ctx.enter_context(tc.tile_pool(name="ex", bufs=6))
    sn_pool = ctx.enter_context(tc.tile_pool(name="sn", bufs=3))
    rc_pool = ctx.enter_context(tc.tile_pool(name="rc", bufs=3))

    # ---- persistent SBUF ----
    wgt_sb = singles.tile([128, 8, KP], BF)            # [c-in-tile, ci, k]
    nc.sync.dma_start(out=wgt_sb, in_=wgt.rearrange("(ci p) k -> p ci k", p=128))
    wg_sb = singles.tile([KP, C], BF)
    nc.sync.dma_start(out=wg_sb, in_=wg)
    ident = singles.tile([128, 128], BF)
    make_identity(nc, ident)
    sel = singles.tile([64, 128], F32)                 # den -> per-head row broadcast
    nc.vector.memset(sel, 0.0)
    nc.vector.memset(sel[0:1, 0:64], 1.0)
    nc.vector.memset(sel[32:33, 64:128], 1.0)
    dn = singles.tile([64, NQB], F32)                  # dens: head A row 0, head B row 32
    nc.vector.memset(dn, 0.0)
    ztuT = singles.tile([128, B * N], BF)              # [k-chan, b*N + n]
    ztuN = singles.tile([128, NTILES, 2 * (DH + 1)], BF)  # [n-in-tile, nt, [vA|1|vB|1]]
    nc.vector.memset(ztuN[:, :, DH:DH + 1], 1.0)
    nc.vector.memset(ztuN[:, :, 2 * DH + 1:2 * DH + 2], 1.0)

    # ---- phase 1: proj1 (ztuT = W_g @ ZT^T) + phase 1.5: transposes (ztuN) ----
    for b in range(B):
        zin = []
        for ci in range(8):
            z = zin_pool.tile([128, N], BF, tag="zin")
            nc.sync.dma_start(out=z, in_=ztt[b, ci * 128:(ci + 1) * 128, :])
            zin.append(z)
        for jn in range(N // NQB):
            p1 = sc_pool.tile([128, 2 * NQB], F32, tag="sc")
            p1v = p1[:, 0:NQB]
            for ci in range(8):
                nc.tensor.matmul(
                    p1v, lhsT=wgt_sb[:, ci, :],
                    rhs=zin[ci][:, jn * NQB:(jn + 1) * NQB],
                    start=(ci == 0), stop=(ci == 7),
                )
            nc.vector.tensor_copy(
                out=ztuT[:, b * N + jn * NQB: b * N + (jn + 1) * NQB], in_=p1v)
        for ntl in range(NTB):
            nt = b * NTB + ntl
            for hh in range(HPER):
                pt = av_pool.tile([128, NQB], BF, tag="av")
                nc.tensor.transpose(
                    pt[:, 0:DH],
                    ztuT[hh * DH:(hh + 1) * DH, nt * NKT:(nt + 1) * NKT],
                    ident[hh * DH:(hh + 1) * DH, hh * DH:(hh + 1) * DH],
                )
                nc.vector.tensor_copy(
                    out=ztuN[:, nt, hh * (DH + 1): hh * (DH + 1) + DH],
                    in_=pt[:, 0:DH])

    # ---- phase 2: attention + proj2, software-pipelined across q-blocks ----
    # Emit q-block j's QK/exp/AV before q-block j-1's normalize+proj2 so the
    # PE queue (in-order) never stalls on the DVE normalization chain.
    def attention_block(b, jq):
        q0 = b * N + jq * NQB
        avs = []
        for hh in range(HPER):
            av = av_pool.tile([DH + 1, NQB], F32, tag="av")
            qT = ztuT[hh * DH:(hh + 1) * DH, q0:q0 + NQB]
            for g in range(NTB // 2):           # groups of 2 key tiles
                sc = sc_pool.tile([128, 2 * NQB], F32, tag="sc")
                ex = ex_pool.tile([128, 2 * NQB], BF, tag="ex")
                for u in range(2):
                    ik = 2 * g + u
                    kT = ztuT[hh * DH:(hh + 1) * DH,
                              b * N + ik * NKT: b * N + (ik + 1) * NKT]
                    nc.tensor.matmul(sc[:, u * NQB:(u + 1) * NQB],
                                     lhsT=kT, rhs=qT, start=True, stop=True)
                nc.scalar.activation(
                    out=ex, in_=sc,
                    func=mybir.ActivationFunctionType.Exp, scale=SCALE)
                for u in range(2):
                    ik = 2 * g + u
                    vT = ztuN[:, b * NTB + ik,
                              hh * (DH + 1): (hh + 1) * (DH + 1)]
                    nc.tensor.matmul(av, lhsT=vT,
                                     rhs=ex[:, u * NQB:(u + 1) * NQB],
                                     start=(ik == 0), stop=(ik == NTB - 1))
            avs.append(av)
        return avs

    def finish_block(b, jq, avs):
        # softmax denominators -> per-head broadcast -> reciprocal -> scale
        nc.vector.tensor_copy(out=dn[0:1, :], in_=avs[0][DH:DH + 1, :])
        nc.vector.tensor_copy(out=dn[32:33, :], in_=avs[1][DH:DH + 1, :])
        bc = sc_pool.tile([128, 2 * NQB], F32, tag="sc")
        nc.tensor.matmul(bc[:, 0:NQB], lhsT=sel, rhs=dn, start=True, stop=True)
        rc = rc_pool.tile([128, NQB], F32)
        nc.vector.reciprocal_approx_fast(out=rc, in_=bc[:, 0:NQB])
        sn = sn_pool.tile([128, NQB], BF)
        nc.vector.tensor_tensor(
            out=sn[0:64, :], in0=avs[0][0:DH, :], in1=rc[0:64, :],
            op=mybir.AluOpType.mult)
        nc.vector.tensor_tensor(
            out=sn[64:128, :], in0=avs[1][0:DH, :], in1=rc[64:128, :],
            op=mybir.AluOpType.mult)

        # proj2: out[q, :] += ssa_norm_g @ W_g  (both heads contracted)
        for t in range(NQB // 128):
            for ch in range(2):
                p2 = av_pool.tile([128, 512], F32, tag="av")
                nc.tensor.matmul(
                    p2, lhsT=sn[:, t * 128:(t + 1) * 128],
                    rhs=wg_sb[:, ch * 512:(ch + 1) * 512],
                    start=True, stop=True)
                p2s = rc_pool.tile([128, 512], F32, tag="p2s")
                nc.vector.tensor_copy(out=p2s, in_=p2)
                r0 = b * N + jq * NQB + t * 128
                nc.sync.dma_start(
                    out=out[r0:r0 + 128, ch * 512:(ch + 1) * 512], in_=p2s)

    pending = None
    for b in range(B):
        for jq in range(N // NQB):
            avs = attention_block(b, jq)
            if pending is not None:
                finish_block(*pending)
            pending = (b, jq, avs)
    finish_block(*pending)


def _get_nc():
    if "nc" not in _CACHE:
        _CACHE["nc"] = _build_kernel()
    return _CACHE["nc"]


def kernel(ZT: np.ndarray, W: np.ndarray) -> np.ndarray:
    ZT = np.asarray(ZT, dtype=np.float32)
    W = np.asarray(W, dtype=np.float32)
    ztt = np.ascontiguousarray(ZT.transpose(0, 2, 1)).astype(ml_dtypes.bfloat16)
    in_maps = []
    for c in range(8):
        wgf = W[c * KP:(c + 1) * KP, :]
        in_maps.append({
            "ztt": ztt,
            "wgt": np.ascontiguousarray(wgf.T).astype(ml_dtypes.bfloat16),
            "wg": np.ascontiguousarray(wgf).astype(ml_dtypes.bfloat16),
        })
    nc = _get_nc()
    res = run_bass_kernel_spmd(nc, in_maps, core_ids=list(range(8)))
    acc = np.zeros((B * N, C), dtype=np.float32)
    for r in res.results:
        acc += r["out"]
    return acc.reshape(B, N, C)


if __name__ == "__main__":
    rng = np.random.default_rng(0)
    zt = rng.standard_normal((B, N, C), dtype=np.float32)
    w = rng.standard_normal((KP * 8, C), dtype=np.float32) * C ** -0.5
    o = kernel(zt, w)
    print("out", o.shape, o.dtype, float(np.abs(o).mean()))


# revision 17
# speedup vs baseline: 1.3828x; 1.1820x over previous
"""Trainium2 Bass kernel for nn_Attention_Encode (B=4, N=2048, DIM=1024, H=16, DH=64).

Sharding: 16 heads -> 8 cores x 2 heads (tensor parallel). Each core computes
  ztu_g = W_g @ ZT^T          (its 128 output channels = 2 heads)
  attention per (batch, head) with Q=K=V=ztu
  partial_out = ssa_g @ W_g   (row-sharded output projection)
Host sums the 8 partials (the all-reduce step of a row-sharded projection).

On-device layout is fully transposed ("scoresT" = [keys, queries]) so that
softmax needs no transposes: the AV matmul's stationary operand [V | ones]
produces both the numerator and the softmax denominator.
"""
import sys

for _p in ('/opt/trn_rl_repo',):
    if _p not in sys.path:
        sys.path.insert(0, _p)

from contextlib import ExitStack

import numpy as np
import ml_dtypes

import concourse.bacc as bacc
import concourse.mybir as mybir
import concourse.tile as tile
from concourse.bass_utils import run_bass_kernel_spmd
from concourse.masks import make_identity

B, N, C = 4, 2048, 1024          # batch, seq, model dim
KP, DH, HPER = 128, 64, 2        # per-core channels, head dim, heads per core
NQB = 512                        # query block
NKT = 128                        # key tile
NTB = N // NKT                   # 16 key tiles per batch
NTILES = B * NTB                 # 64 n-tiles total
SCALE = DH ** -0.5               # 0.125
BF = mybir.dt.bfloat16
F32 = mybir.dt.float32
F32R = mybir.dt.float32r

_CACHE = {}


def _build_kernel():
    nc = bacc.Bacc("TRN2", target_bir_lowering=False, debug=False)
    ztt = nc.dram_tensor("ztt", [B, C, N], BF, kind="ExternalInput").ap()
    wgt = nc.dram_tensor("wgt", [C, KP], BF, kind="ExternalInput").ap()   # W_g^T
    wg = nc.dram_tensor("wg", [KP, C], BF, kind="ExternalInput").ap()     # W_g
    out = nc.dram_tensor("out", [B * N, C], F32, kind="ExternalOutput").ap()

    with tile.TileContext(nc) as tc, ExitStack() as ctx:
        _body(ctx, tc, ztt, wgt, wg, out)
    nc.compile()
    return nc


def _body(ctx, tc, ztt, wgt, wg, out):
    nc = tc.nc
    singles = ctx.enter_context(tc.tile_pool(name="singles", bufs=1))
    zin_pool = ctx.enter_context(tc.tile_pool(name="zin", bufs=8))
    sc_pool = ctx.enter_context(tc.tile_pool(name="sc", bufs=2, space="PSUM"))
    av_pool = ctx.enter_context(tc.tile_pool(name="av", bufs=2, space="PSUM"))
    p2_pool = ctx.enter_context(tc.tile_pool(name="p2", bufs=2, space="PSUM"))
    ex_pool = ctx.enter_context(tc.tile_pool(name="ex", bufs=8))
    sn_pool = ctx.enter_context(tc.tile_pool(name="sn", bufs=3))
    rc_pool = ctx.enter_context(tc.tile_pool(name="rc", bufs=3))

    # ---- persistent SBUF ----
    wgt_sb = singles.tile([128, 8, KP], BF)            # [c-in-tile, ci, k]
    nc.sync.dma_start(out=wgt_sb, in_=wgt.rearrange("(ci p) k -> p ci k", p=128))
    wg_sb = singles.tile([KP, C], BF)
    nc.sync.dma_start(out=wg_sb, in_=wg)
    ident = singles.tile([128, 128], BF)
    make_identity(nc, ident)
    self_f = singles.tile([64, 128], F32)
    nc.vector.memset(self_f, 0.0)
    nc.vector.memset(self_f[0:1, 0:64], 1.0)
    nc.vector.memset(self_f[32:33, 64:128], 1.0)
    sel = singles.tile([64, 128], F32R)                # den -> per-head row broadcast
    nc.vector.tensor_copy(out=sel, in_=self_f)
    dn = singles.tile([64, NQB], F32R)                 # dens: head A row 0, head B row 32
    nc.vector.memset(dn[:].bitcast(F32), 0.0)
    ztuT = singles.tile([128, B * N], BF)              # [k-chan, b*N + n]
    ztuN = singles.tile([128, NTILES, 2 * (DH + 1)], BF)  # [n-in-tile, nt, [vA|1|vB|1]]
    nc.vector.memset(ztuN[:, :, DH:DH + 1], 1.0)
    nc.vector.memset(ztuN[:, :, 2 * DH + 1:2 * DH + 2], 1.0)

    # ---- phase 1: proj1 (ztuT = W_g @ ZT^T) + phase 1.5: transposes (ztuN) ----
    for b in range(B):
        zin = []
        for ci in range(8):
            z = zin_pool.tile([128, N], BF, tag="zin")
            nc.sync.dma_start(out=z, in_=ztt[b, ci * 128:(ci + 1) * 128, :])
            zin.append(z)
        for jn in range(N // NQB):
            p1 = sc_pool.tile([128, 2 * NQB], F32, tag="sc")
            p1v = p1[:, 0:NQB]
            for ci in range(8):
                nc.tensor.matmul(
                    p1v, lhsT=wgt_sb[:, ci, :],
                    rhs=zin[ci][:, jn * NQB:(jn + 1) * NQB],
                    start=(ci == 0), stop=(ci == 7),
                )
            nc.vector.tensor_copy(
                out=ztuT[:, b * N + jn * NQB: b * N + (jn + 1) * NQB], in_=p1v)
        for ntl in range(NTB):
            nt = b * NTB + ntl
            for hh in range(HPER):
                pt = av_pool.tile([128, NQB], BF, tag="av")
                nc.tensor.transpose(
                    pt[:, 0:DH],
                    ztuT[hh * DH:(hh + 1) * DH, nt * NKT:(nt + 1) * NKT],
                    ident[hh * DH:(hh + 1) * DH, hh * DH:(hh + 1) * DH],
                )
                nc.vector.tensor_copy(
                    out=ztuN[:, nt, hh * (DH + 1): hh * (DH + 1) + DH],
                    in_=pt[:, 0:DH])

    # ---- phase 2: attention + proj2, software-pipelined across q-blocks ----
    # Emit q-block j's QK/exp/AV before q-block j-1's normalize+proj2 so the
    # PE queue (in-order) never stalls on the DVE normalization chain.
    def attention_block(b, jq):
        # Heads interleaved per group: head A's QK runs on PE array row-tile
        # T0 (partitions 0-63) and head B's on T8 (64-127) back-to-back, so
        # the two 64-row matmuls can overlap in the array. QK (64x128 mode)
        # and AV (128x128 mode) are batched to limit tiling-mode switches.
        q0 = b * N + jq * NQB
        avs = [av_pool.tile([DH + 1, NQB], F32, tag="av", name=f"av{h}")
               for h in range(HPER)]
        for g in range(NTB // 2):               # groups of 2 key tiles
            scs, exs = [], []
            for hh in range(HPER):
                sc = sc_pool.tile([128, 2 * NQB], F32, tag="sc")
                qT = ztuT[hh * DH:(hh + 1) * DH, q0:q0 + NQB]
                for u in range(2):
                    ik = 2 * g + u
                    kT = ztuT[hh * DH:(hh + 1) * DH,
                              b * N + ik * NKT: b * N + (ik + 1) * NKT]
                    nc.tensor.matmul(sc[:, u * NQB:(u + 1) * NQB],
                                     lhsT=kT, rhs=qT, start=True, stop=True)
                scs.append(sc)
            for hh in range(HPER):
                ex = ex_pool.tile([128, 2 * NQB], BF, tag="ex")
                nc.scalar.activation(
                    out=ex, in_=scs[hh],
                    func=mybir.ActivationFunctionType.Exp, scale=SCALE)
                exs.append(ex)
            for hh in range(HPER):
                for u in range(2):
                    ik = 2 * g + u
                    vT = ztuN[:, b * NTB + ik,
                              hh * (DH + 1): (hh + 1) * (DH + 1)]
                    nc.tensor.matmul(avs[hh], lhsT=vT,
                                     rhs=exs[hh][:, u * NQB:(u + 1) * NQB],
                                     start=(ik == 0), stop=(ik == NTB - 1))
        return avs

    def finish_block(b, jq, avs):
        # softmax denominators -> per-head broadcast -> reciprocal -> scale
        nc.vector.tensor_copy(out=dn[0:1, :], in_=avs[0][DH:DH + 1, :])
        nc.vector.tensor_copy(out=dn[32:33, :], in_=avs[1][DH:DH + 1, :])
        bc = p2_pool.tile([128, 512], F32, tag="p2")
        nc.tensor.matmul(bc, lhsT=sel, rhs=dn, start=True, stop=True)
        rc = rc_pool.tile([128, NQB], F32)
        nc.vector.reciprocal_approx_fast(out=rc, in_=bc)
        sn = sn_pool.tile([128, NQB], BF)
        nc.vector.tensor_tensor(
            out=sn[0:64, :], in0=avs[0][0:DH, :], in1=rc[0:64, :],
            op=mybir.AluOpType.mult)
        nc.vector.tensor_tensor(
            out=sn[64:128, :], in0=avs[1][0:DH, :], in1=rc[64:128, :],
            op=mybir.AluOpType.mult)

        # proj2: out[q, :] += ssa_norm_g @ W_g  (both heads contracted)
        for t in range(NQB // 128):
            for ch in range(2):
                p2 = p2_pool.tile([128, 512], F32, tag="p2")
                nc.tensor.matmul(
                    p2, lhsT=sn[:, t * 128:(t + 1) * 128],
                    rhs=wg_sb[:, ch * 512:(ch + 1) * 512],
                    start=True, stop=True)
                p2s = rc_pool.tile([128, 512], F32, tag="p2s")
                nc.vector.tensor_copy(out=p2s, in_=p2)
                r0 = b * N + jq * NQB + t * 128
                nc.sync.dma_start(
                    out=out[r0:r0 + 128, ch * 512:(ch + 1) * 512], in_=p2s)

    pending = None
    for b in range(B):
        for jq in range(N // NQB):
            avs = attention_block(b, jq)
            if pending is not None:
                finish_block(*pending)
            pending = (b, jq, avs)
    finish_block(*pending)


def _get_nc():
    if "nc" not in _CACHE:
        _CACHE["nc"] = _build_kernel()
    return _CACHE["nc"]


def kernel(ZT: np.ndarray, W: np.ndarray) -> np.ndarray:
    ZT = np.asarray(ZT, dtype=np.float32)
    W = np.asarray(W, dtype=np.float32)
    ztt = np.ascontiguousarray(ZT.transpose(0, 2, 1)).astype(ml_dtypes.bfloat16)
    in_maps = []
    for c in range(8):
        wgf = W[c * KP:(c + 1) * KP, :]
        in_maps.append({
            "ztt": ztt,
            "wgt": np.ascontiguousarray(wgf.T).astype(ml_dtypes.bfloat16),
            "wg": np.ascontiguousarray(wgf).astype(ml_dtypes.bfloat16),
        })
    nc = _get_nc()
    res = run_bass_kernel_spmd(nc, in_maps, core_ids=list(range(8)))
    acc = np.zeros((B * N, C), dtype=np.float32)
    for r in res.results:
        acc += r["out"]
    return acc.reshape(B, N, C)


if __name__ == "__main__":
    rng = np.random.default_rng(0)
    zt = rng.standard_normal((B, N, C), dtype=np.float32)
    w = rng.standard_normal((KP * 8, C), dtype=np.float32) * C ** -0.5
    o = kernel(zt, w)
    print("out", o.shape, o.dtype, float(np.abs(o).mean()))
